# revision 20
# baseline (speedup 1.0000x reference)
"""Chunked-pipeline causal average pooling (AverageContext) Trainium2 kernel.

out[b, t, c] = mean_{s<=t} x[b, s, c]  for x [16, 4096, 128] fp32.
Data-parallel over batch: 2 batches per NeuronCore across 8 cores.

T is split into CH=8 chunks of 512 rows (t = 512*k + 4*p + j), pipelined so
the DMA engines stream continuously: the 8MB in+out round trip at ~360 GB/s
per core is the 23.3us floor, and the DMA queue runs gapless from first to
last byte. Per chunk:

  1. in-DMA  x3[p, j, c]  (2KB descriptors). ALL 16 in-DMAs of a rep are
     emitted before any compute so SP's in-order queue -- which also hosts
     the out-DMAs -- never stalls the prefetch (xp bufs=16).
  2. PE: J=4 accumulating matmuls  ps[:, J-1, :] += trilI.T @ x3[:, j, :]
     (inclusive triangular, f32r self-loading) give B'[p] = the inclusive
     cumsum at slot J-1; for k>0 one more accumulate  sel127.T @ carry
     broadcasts the previous chunk's running total from partition 127
     (sel127 row 127 = ones) -- the carry never leaves the chip.
  3. ACT copies the B' column PSUM->SBUF: for b0 a small ctmp tile (next
     chunk's carry); for b1 slot J-1 of an SBUF tile sb, which seeds its
     chain AND doubles as the carry (GPSIMD cannot access PSUM).
  4. remaining slots by DOWNWARD SUBTRACTION (3 serial [128,128] ops):
       v[j] = v[j+1] - x3[:, j+1, :]   -- b0 on DVE in PSUM, b1 on Pool in
     SBUF.
  5. scale by inv[p, 4k+j]: b0 as ONE wide DVE tensor_mul (stride-0-over-c
     broadcast AP); b1 split: ACT muls the HIGH slots (slot 3 is ready on
     ACT's own queue right after the seed copy, before the chain runs) and
     Pool wide-muls the low slots at the chain's end.
  6. out-DMA from SP. Batch order (b0, b1) puts the slower queue first.

trilI / sel127 / invt are GENERATED ON-CHIP (gpsimd memset + affine_select
+ iota, DVE reciprocal) during the DMA lead-in when Pool/DVE idle, so no
const bytes ride the DMA stream. (memset/affine_select cannot target f32r:
generate in f32 scratch, tensor_copy to f32r. affine_select codegen only
implements is_ge/is_gt.)

Cost-model timing: single execution 27759 ns (lead-in 2.3us + stream
23.3us + 0.7us tail gaps + 1.4us barrier), steady-state marginal
23798 ns/iter vs the 23296 ns DMA floor. The staged v1 baseline: 68599 ns
sim / 78709 ns measured on the harness rig.
"""

import os
import sys

import numpy as np

for _p in (
    "/opt/trn_rl_repo",
    "/root/.axon_site",
    "/root/.axon_site/_ro/trn_rl_repo",
    "/root/.axon_site/_ro/pypackages",
):
    if os.path.isdir(_p) and _p not in sys.path:
        sys.path.append(_p)

import concourse.bass as bass  # noqa: E402
import concourse.mybir as mybir  # noqa: E402
import concourse.tile as tile  # noqa: E402

B, T, C = 16, 4096, 128
NCORES = 8
BPC = B // NCORES
P = 128
CH = 8  # chunks per batch (CHR = T/CH rows per chunk, J = CHR/128 j-slots;
#         J*C*4 bytes is the DMA descriptor size -- keep >= 2KB)
# engine assignment knobs: per batch index, which engine runs the ctmp copy
# and the wide scale ("dve", "pool", "act")
B1_ACT_SCALE = 2  # b1 scale slots [0, n) issued as small ACT muls instead of Pool
B0_ACT_SCALE = 0  # b0 scale slots [0, n) issued as small ACT muls instead of DVE
B1_ACT_HI = True  # ACT takes b1's high j-slots (ready early) instead of low
BORDER = (0, 1)  # per-chunk batch emission order
GEN_CONSTS = True  # generate trilI/sel127/invt on-chip (Pool idles in the lead-in)
ROTATE_OUTS = False  # emit chunk 0's out-DMAs last (they are long ready)
OUT_B0_FIRST = False  # emit each chunk-pair's out-DMAs as (b0, b1) after both bodies
B0_CTMP = "act"   # engine for b0's carry-column copy ("act"/"dve")
B1_SEED = "act"   # engine for b1's PSUM->SBUF seed copy ("act"/"dve")
OUT_ON_SP = True  # issue out-DMAs from SP (each rep's in-DMAs are emitted first)
XP_BUFS = 16
OP_BUFS = 8
PSUM_BUFS = 6
def _geom():
    chr_ = T // CH
    return chr_, chr_ // P

F32 = mybir.dt.float32
F32R = mybir.dt.float32r


def _legalize_sync_waits(nc):
    """Move excess sync waits onto standalone InstEventSemaphore instructions.

    Walrus on this stack rejects instructions with more than one sync wait,
    and the fp32/f32r self-loading matmul rejects even one (waits land on the
    LW slot).
    """
    uid = [0]

    def mk_wait(engine, w):
        uid[0] += 1
        return mybir.InstEventSemaphore(
            name=f"I-waitfix-{uid[0]}",
            engine=engine,
            ins=[],
            outs=[],
            sync_info=mybir.SyncInfo(on_wait=[w], on_update=[]),
        )

    for f in nc.m.functions:
        for blk in f.blocks:
            out = []
            for inst in blk.instructions:
                si = inst.sync_info
                waits = list(si.on_wait) if si is not None and si.on_wait else []
                keep = 0 if type(inst).__name__ in ("InstMatmult", "InstLdweights") else 1
                if len(waits) > keep:
                    moved = waits[: len(waits) - keep] if keep else waits
                    kept = waits[len(waits) - keep :] if keep else []
                    for w in moved:
                        out.append(mk_wait(inst.engine, w))
                    inst.sync_info = mybir.SyncInfo(
                        on_wait=kept,
                        on_update=list(si.on_update) if si.on_update else [],
                    )
                out.append(inst)
            blk.instructions = out


def _build_nc(legalize=True, reps=1):
    from contextlib import ExitStack

    nc = bass.Bass()
    # F32R so the in-DMA into the f32r x3 tile (matmul rhs) is cast-free;
    # bit-identical to f32 (dt.np maps it back to np.float32)
    x_in = nc.declare_dram_parameter("x", [BPC, T, C], F32R, isOutput=False)
    if not GEN_CONSTS:
        # trilI | sel127 packed into one f32r tensor; invt separate (the ACT
        # tensor_scalar's scale AP must be plain FP32)
        co_in = nc.declare_dram_parameter("consts", [P, 2 * P], F32R, isOutput=False)
        inv_in = nc.declare_dram_parameter("invt", [P, T // P], F32, isOutput=False)
    # also F32R (bit-identical): rep r>0 re-reads it into f32r x3 tiles
    y_out = nc.declare_dram_parameter("out", [BPC, T, C], F32R, isOutput=True)

    CHR, J = _geom()
    with tile.TileContext(nc) as tc, ExitStack() as ctx:
        consts = ctx.enter_context(tc.tile_pool(name="consts", bufs=1))
        xp = ctx.enter_context(tc.tile_pool(name="xp", bufs=XP_BUFS))
        op = ctx.enter_context(tc.tile_pool(name="op", bufs=OP_BUFS))
        cp = ctx.enter_context(tc.tile_pool(name="cp", bufs=4))
        sp_ = ctx.enter_context(tc.tile_pool(name="sp_", bufs=4))
        pp = ctx.enter_context(tc.tile_pool(name="pp", bufs=PSUM_BUFS, space="PSUM"))

        def chunk_dram(t, b, k):
            return t[b].rearrange("(k p j) c -> k p (j c)", k=CH, p=P)[k]

        # first two x in-DMAs go before the const loads: the consts are not
        # needed until the first matmul, and leading with them would idle the
        # DMA engines for ~2us
        x3_first = []
        for b in range(BPC):
            x3f = xp.tile([P, J, C], F32R, tag="x3")
            nc.sync.dma_start(
                out=x3f,
                in_=chunk_dram(x_in, b, 0).rearrange("p (j c) -> p j c", c=C),
            )
            x3_first.append(x3f)

        if GEN_CONSTS:
            # generate the constants on-chip while the x in-DMAs stream:
            # Pool/DVE are idle for the first ~3.5us and the DMA queue
            # carries 455 fewer ns
            # memset/affine_select cannot target f32r, so generate in f32
            # scratch and tensor_copy into the f32r matmul operands
            tril_f = consts.tile([P, P], F32, tag="tril_f")
            nc.gpsimd.memset(tril_f, 0.0)
            # iota = k - m; keep 0 where k > m, fill 1 where k <= m
            nc.gpsimd.affine_select(
                out=tril_f, in_=tril_f, compare_op=mybir.AluOpType.is_gt,
                fill=1.0, base=0, channel_multiplier=1, pattern=[[-1, P]],
            )
            trilI = consts.tile([P, P], F32R, tag="trilI")
            nc.gpsimd.tensor_copy(trilI, tril_f)
            sel_f = consts.tile([P, P], F32, tag="sel_f")
            nc.gpsimd.memset(sel_f, 0.0)
            # iota = 126 - k; keep 0 where k <= 126, fill 1 at k == 127
            nc.gpsimd.affine_select(
                out=sel_f, in_=sel_f, compare_op=mybir.AluOpType.is_ge,
                fill=1.0, base=P - 2, channel_multiplier=-1, pattern=[[0, P]],
            )
            sel127 = consts.tile([P, P], F32R, tag="sel127")
            nc.gpsimd.tensor_copy(sel127, sel_f)
            invi = consts.tile([P, T // P], mybir.dt.int32, tag="invi")
            # value(p, k, j) = 1 + J*p + CHR*k + j = t + 1
            nc.gpsimd.iota(
                invi, pattern=[[CHR, CH], [1, J]], base=1, channel_multiplier=J
            )
            invf = consts.tile([P, T // P], F32, tag="invf")
            nc.gpsimd.tensor_copy(invf, invi)
            invt = consts.tile([P, T // P], F32, tag="invt")
            nc.vector.reciprocal(invt, invf)
        else:
            # const loads on ACT's otherwise-idle HWDGE: SP's gen pipeline
            # then feeds x in-DMAs exclusively
            cpack = consts.tile([P, 2 * P], F32R, tag="cpack")
            nc.scalar.dma_start(out=cpack, in_=co_in[:, :])
            trilI = cpack[:, 0:P]
            sel127 = cpack[:, P : 2 * P]
            invt = consts.tile([P, T // P], F32, tag="invt")
            nc.scalar.dma_start(out=invt, in_=inv_in[:, :])

        for r in range(reps):
            # emit the whole rep's in-DMAs first: they carry no waits within
            # the rep (cross-rep serialization comes from reading y_out), so
            # SP can also host the out-DMAs without stalling the prefetch
            x3s = {}
            for k in range(CH):
                for b in BORDER:
                    if r == 0 and k == 0:
                        x3s[(k, b)] = x3_first[b]
                        continue
                    src_t = x_in if r == 0 else y_out
                    x3 = xp.tile([P, J, C], F32R, tag="x3")
                    nc.sync.dma_start(
                        out=x3,
                        in_=chunk_dram(src_t, b, k).rearrange(
                            "p (j c) -> p j c", c=C
                        ),
                    )
                    x3s[(k, b)] = x3

            carry = [None] * BPC
            pending_outs = []
            for k in range(CH):
                outs = {}
                border = BORDER
                for b in border:
                    x3 = x3s[(k, b)]

                    # inclusive prefix at slot J-1 via accumulating matmuls
                    ps = pp.tile([P, J, C], F32, tag="ps")
                    for j in range(J):
                        nc.tensor.matmul(
                            ps[:, J - 1 : J, :],
                            trilI,
                            x3[:, j, :],
                            start=(j == 0),
                            stop=(j == J - 1 and k == 0),
                        )
                    if k > 0:
                        # carry broadcast ON the PE: sel127[k, m] = (k==127)
                        # picks the previous chunk's inclusive total from
                        # partition 127 of its SBUF copy and accumulates it
                        # across all partitions -- no DMA, no partition hop
                        nc.tensor.matmul(
                            ps[:, J - 1 : J, :],
                            sel127,
                            carry[b],
                            start=False,
                            stop=True,
                        )

                    out_t = op.tile([P, J, C], F32R, tag="out_t")
                    inv_b = bass.AP(
                        tensor=invt.tensor,
                        offset=invt.offset + k * J,
                        ap=[invt.ap[0], [1, J], [0, C]],
                    )
                    if b == 0:
                        if k < CH - 1:
                            ctmp = cp.tile([P, C], F32R, tag="ctmp")
                            if B0_CTMP == "act":
                                nc.scalar.copy(ctmp, ps[:, J - 1, :])
                            else:
                                nc.vector.tensor_copy(ctmp, ps[:, J - 1, :])
                            carry[b] = ctmp
                        for j in range(J - 2, -1, -1):
                            nc.vector.tensor_sub(
                                ps[:, j, :], ps[:, j + 1, :], x3[:, j + 1, :]
                            )
                        na0 = B0_ACT_SCALE
                        if na0 < J:
                            inv_hi0 = bass.AP(
                                tensor=invt.tensor,
                                offset=invt.offset + k * J + na0,
                                ap=[invt.ap[0], [1, J - na0], [0, C]],
                            )
                            nc.vector.tensor_mul(
                                out_t[:, na0:J, :], ps[:, na0:J, :], inv_hi0
                            )
                        for j in range(na0 - 1, -1, -1):
                            col = k * J + j
                            nc.scalar.mul(
                                out_t[:, j, :], ps[:, j, :], invt[:, col : col + 1]
                            )
                    else:
                        sb = sp_.tile([P, J, C], F32R, tag="sb")
                        if B1_SEED == "act":
                            nc.scalar.copy(sb[:, J - 1, :], ps[:, J - 1, :])
                        else:
                            nc.vector.tensor_copy(sb[:, J - 1, :], ps[:, J - 1, :])
                        for j in range(J - 2, -1, -1):
                            nc.gpsimd.tensor_sub(
                                sb[:, j, :], sb[:, j + 1, :], x3[:, j + 1, :]
                            )
                        na = B1_ACT_SCALE
                        if B1_ACT_HI:
                            # ACT takes the HIGH slots: slot J-1 is ready on
                            # ACT's own queue right after the seed copy, and
                            # Pool's wide op covers the chain-end slots
                            for j in range(J - na, J):
                                col = k * J + j
                                nc.scalar.mul(
                                    out_t[:, j, :], sb[:, j, :], invt[:, col : col + 1]
                                )
                            if na < J:
                                inv_lo = bass.AP(
                                    tensor=invt.tensor,
                                    offset=invt.offset + k * J,
                                    ap=[invt.ap[0], [1, J - na], [0, C]],
                                )
                                nc.gpsimd.tensor_mul(
                                    out_t[:, 0 : J - na, :], sb[:, 0 : J - na, :], inv_lo
                                )
                        else:
                            if na < J:
                                inv_hi = bass.AP(
                                    tensor=invt.tensor,
                                    offset=invt.offset + k * J + na,
                                    ap=[invt.ap[0], [1, J - na], [0, C]],
                                )
                                nc.gpsimd.tensor_mul(
                                    out_t[:, na:J, :], sb[:, na:J, :], inv_hi
                                )
                            for j in range(na - 1, -1, -1):
                                col = k * J + j
                                nc.scalar.mul(
                                    out_t[:, j, :], sb[:, j, :], invt[:, col : col + 1]
                                )
                        if k < CH - 1:
                            carry[b] = sb[:, J - 1, :]
                    outs[b] = out_t

                pending_outs.append(
                    (k, [(b, outs[b]) for b in ((0, 1) if OUT_B0_FIRST else border)])
                )
                oeng = nc.sync if OUT_ON_SP else nc.scalar
                if ROTATE_OUTS:
                    # hold chunk 0's outs until the end: when the stream's
                    # tail arrives they are long since compute-ready, so the
                    # final transfers carry no compute wait
                    emit_now = [po for po in pending_outs if po[0] != 0] if k < CH - 1 else pending_outs
                else:
                    emit_now = pending_outs
                for kk, obs in emit_now:
                    for b, ot in obs:
                        oeng.dma_start(
                            out=chunk_dram(y_out, b, kk),
                            in_=ot.rearrange("p j c -> p (j c)"),
                        )
                pending_outs = [po for po in pending_outs if po not in emit_now]

    if legalize:
        _legalize_sync_waits(nc)
    return nc


def _make_consts():
    if GEN_CONSTS:
        return {}
    # matmul computes out[m] = sum_k lhsT[k, m] * rhs[k]; we want sum_{k<=m}:
    # lhsT[k, m] = 1 iff k <= m  -> upper triangular INCLUSIVE as stored [k, m]
    trilI = np.triu(np.ones((P, P), dtype=np.float32), 0)
    sel127 = np.zeros((P, P), dtype=np.float32)
    sel127[P - 1, :] = 1.0
    CHR, J = _geom()
    k_idx = np.arange(CH)[None, :, None]
    p_idx = np.arange(P)[:, None, None]
    j_idx = np.arange(J)[None, None, :]
    t_idx = CHR * k_idx + J * p_idx + j_idx
    invt = (1.0 / (t_idx + 1.0)).astype(np.float32).reshape(P, T // P)
    return dict(consts=np.concatenate([trilI, sel127], axis=1), invt=invt)


_NC = None


def _get_nc():
    global _NC
    if _NC is None:
        _NC = _build_nc()
    return _NC


def kernel(x: np.ndarray) -> np.ndarray:
    from concourse.bass_utils import run_bass_kernel_spmd

    assert x.shape == (B, T, C), x.shape
    x = np.asarray(x, dtype=np.float32)
    nc = _get_nc()
    consts = _make_consts()
    in_maps = [
        {"x": np.ascontiguousarray(x[i * BPC : (i + 1) * BPC]), **consts}
        for i in range(NCORES)
    ]
    res = run_bass_kernel_spmd(nc, in_maps, list(range(NCORES))).results
    return np.concatenate([res[i]["out"] for i in range(NCORES)], axis=0).astype(
        np.float32
    )


if __name__ == "__main__":
    x = np.random.randn(B, T, C).astype(np.float32)
    y = kernel(x)
    ref = np.cumsum(x, axis=1) / (np.arange(T) + 1.0)[None, :, None]
    err = np.abs(y - ref).max() / np.abs(ref).max()
    print("max abs-rel err:", err)
